# revision 18
# baseline (speedup 1.0000x reference)
"""GQA attention (S=2048, D=2048, 32 q-heads / 8 kv-heads, rope, causal) on 8
Trainium2 NeuronCores, tensor-parallel over heads (1 kv head + 4 q heads per
core), chunked AllToAll re-shard overlapped with compute, row-sharded output.

Self-contained: takes full inputs, shards on host, runs one SPMD Bass/Tile
kernel via run_bass_kernel_spmd, reassembles the full output.

Layout notes (activations on-chip live in the transposed/"T" domain):
 - xT (D,S) host-transposed so the contraction dim D is the SBUF partition dim.
 - q/k weights are column-permuted per head (evens then odds) so rope becomes
   ops on contiguous 32-row blocks; scores are permutation-invariant.
 - scoresT[s,q] = kT.T @ qT per 128-row s-block; softmax denominators come for
   free from a ones-row appended to vT (row 64 of the PV psum after transpose).
 - softmax skips the max-subtraction: scores*0.125 ~ N(0,1), exp is safe in f32.
 - causal masking: s-blocks strictly above the diagonal are skipped, the
   diagonal 128x128 sub-block gets mask[:128,:128].T added pre-exp (all
   diagonal blocks of a causal mask are identical), below-diagonal sub-block
   columns inside partial tiles are zero-filled in probs.
 - matmuls run in bf16 (fast weight load, fp32 psum accumulate); inputs are
   cast on the fly (gpsimd for xT tiles, vector for wo tiles).
"""
import os
import sys
from contextlib import ExitStack

import numpy as np

try:
    import concourse.bass as bass  # noqa: F401
except ImportError:  # platform tree not on sys.path in a fresh dir
    sys.path.insert(0, "/opt/trn_rl_repo")
    import concourse.bass as bass  # noqa: F401

import concourse.mybir as mybir
from concourse import bacc, bass_utils, tile
from concourse.masks import make_identity

F32 = mybir.dt.float32
BF16 = mybir.dt.bfloat16
AF = mybir.ActivationFunctionType

S = 2048          # sequence length
D = 2048          # model dim
HD = 64           # head dim
N_CORES = 8
QH_PER_CORE = 4   # q heads per core (32/8)
QCOLS = QH_PER_CORE * HD      # 256 q-projection cols per core
KVCOLS = 2 * HD               # 128 packed k|v cols per core
ROWS_PER_CORE = S // N_CORES  # 256 output rows per core


def _build():
    nc = bacc.Bacc("TRN2", target_bir_lowering=False, debug=False,
                   num_devices=N_CORES)
    xT_d = nc.dram_tensor("xT", [4, 16, 128, 512], BF16, kind="ExternalInput")
    wq_d = nc.dram_tensor("wq", [128, 16, QCOLS], BF16, kind="ExternalInput")
    wkv_d = nc.dram_tensor("wkv", [128, 16, KVCOLS], BF16, kind="ExternalInput")
    wo_d = nc.dram_tensor("wo", [128, 16, D], BF16, kind="ExternalInput")
    cos_d = nc.dram_tensor("cosT", [HD // 2, S], F32, kind="ExternalInput")
    sin_d = nc.dram_tensor("sinT", [HD // 2, S], F32, kind="ExternalInput")
    mask_d = nc.dram_tensor("maskT01", [128, 128], BF16, kind="ExternalInput")
    out_d = nc.dram_tensor("out", [ROWS_PER_CORE, D], F32, kind="ExternalOutput")

    with tile.TileContext(nc) as tc, ExitStack() as top:
        persist = top.enter_context(tc.tile_pool(name="persist", bufs=1))
        qTs = [persist.tile([HD, S], BF16, name=f"qT{i}", uniquify=False)
               for i in range(QH_PER_CORE)]
        kT = persist.tile([HD, S], BF16, name="kT")
        v128 = persist.tile([128, 16, 128], BF16, name="v128")
        attnT0 = persist.tile([128, S], BF16, name="attnT0")
        attnT1 = persist.tile([128, S], BF16, name="attnT1")
        attnTs = [attnT0, attnT1]
        maskT_sb = persist.tile([128, 128], BF16, name="maskT_sb")
        nc.scalar.dma_start(maskT_sb[:], mask_d.ap())
        # full wo prefetched + cast to bf16 during earlier stages
        wo_sb = persist.tile([128, 16, D], BF16, name="wo_sb")

        dram = top.enter_context(tc.tile_pool(name="dram", bufs=1, space="DRAM"))
        a2a_in = [dram.tile([N_CORES, 128, ROWS_PER_CORE], BF16,
                            name=f"a2a_in{i}", uniquify=False)
                  for i in range(2)]
        a2a_out = [dram.tile([N_CORES, 128, ROWS_PER_CORE], BF16,
                             name=f"a2a_out{i}", uniquify=False)
                   for i in range(2)]

        # ------- Merged pipeline: projections + rope + attention ----------
        # Attention q-tile t only needs data from s-quarters <= t, so its
        # blocks are interleaved into projection quarter t+1: dense projection
        # matmuls fill the latency bubbles of the scores->exp->PV chain.
        with ExitStack() as ctx:
            wpool = ctx.enter_context(tc.tile_pool(name="wpool", bufs=1))
            wq_sb = wpool.tile([128, 16, QCOLS], BF16, name="wq_sb")
            wkv_sb = wpool.tile([128, 16, KVCOLS], BF16, name="wkv_sb")
            cos_sb = wpool.tile([HD // 2, S], F32, name="cos_sb")
            sin_sb = wpool.tile([HD // 2, S], F32, name="sin_sb")
            vT = wpool.tile([HD + 1, S], F32, name="vT")
            identity = wpool.tile([HD + 1, HD + 1], F32, name="identity")
            make_identity(nc, identity[:])
            nc.sync.dma_start(wq_sb[:], wq_d.ap())
            nc.sync.dma_start(wkv_sb[:], wkv_d.ap())
            nc.scalar.dma_start(cos_sb[:], cos_d.ap())
            nc.scalar.dma_start(sin_sb[:], sin_d.ap())

            xtb_pool = ctx.enter_context(tc.tile_pool(name="xtb", bufs=6))
            pacc_pool = ctx.enter_context(
                tc.tile_pool(name="pacc", bufs=3, space="PSUM"))
            psc_pool = ctx.enter_context(
                tc.tile_pool(name="psc", bufs=2, space="PSUM"))
            po_pool = ctx.enter_context(
                tc.tile_pool(name="po", bufs=3, space="PSUM"))
            tmp_pool = ctx.enter_context(tc.tile_pool(name="ropetmp", bufs=2))
            probs_pool = ctx.enter_context(tc.tile_pool(name="probs", bufs=6))
            nrm_pool = ctx.enter_context(tc.tile_pool(name="nrm", bufs=4))

            nc.vector.memset(vT[HD:HD + 1, :], 1.0)
            nc.vector.memset(v128[:, :, HD + 1:], 0.0)

            def rope_pair(dst, dst_cols, src, a_row, cs, sn, tag):
                """dst rows [0:32] = a*cos - b*sin ; rows [32:64] = a*sin+b*cos
                with a = src rows [a_row:a_row+32], b = the next 32 rows."""
                a = src[a_row:a_row + 32, :]
                b = src[a_row + 32:a_row + 64, :]
                t1 = tmp_pool.tile([32, 512], F32, name=f"t1{tag}", tag="t1")
                t2 = tmp_pool.tile([32, 512], F32, name=f"t2{tag}", tag="t2")
                nc.vector.tensor_mul(t1[:], a, cs)
                nc.vector.tensor_mul(t2[:], b, sn)
                nc.vector.tensor_sub(
                    dst[0:32, dst_cols[0]:dst_cols[1]], t1[:], t2[:])
                t3 = tmp_pool.tile([32, 512], F32, name=f"t3{tag}", tag="t3")
                t4 = tmp_pool.tile([32, 512], F32, name=f"t4{tag}", tag="t4")
                nc.vector.tensor_mul(t3[:], a, sn)
                nc.vector.tensor_mul(t4[:], b, cs)
                nc.vector.tensor_add(
                    dst[32:64, dst_cols[0]:dst_cols[1]], t3[:], t4[:])

            def attention_block(h, t, b, nb, po):
                qh = qTs[h]
                j = max(0, b - 4 * t)
                col0 = 128 * j
                psc = psc_pool.tile([128, 512], F32,
                                    name=f"psc{h}{t}{b}", tag="psc")
                nc.tensor.matmul(
                    psc[:, col0:512],
                    kT[:, 128 * b:128 * (b + 1)],
                    qh[:, 512 * t + col0:512 * (t + 1)],
                    start=True, stop=True)
                probs = probs_pool.tile([128, 512], BF16,
                                        name=f"pr{h}{t}{b}", tag="probs")
                nc.scalar.activation(probs[:, col0:512], psc[:, col0:512],
                                     AF.Exp, scale=0.125)
                if b >= 4 * t:
                    # zero the strictly-upper triangle of the diagonal
                    # 128x128 sub-block post-exp (0/1 mask; exp never waits
                    # on the vector engine)
                    nc.vector.tensor_mul(probs[:, col0:col0 + 128],
                                         probs[:, col0:col0 + 128],
                                         maskT_sb[:])
                nc.tensor.matmul(po[:, col0:512], v128[:, b, :],
                                 probs[:, col0:512],
                                 start=(b == 0), stop=(b == nb - 1))

            def finish_tile(h, t, po):
                den = nrm_pool.tile([1, 512], F32, name=f"dn{h}{t}", tag="den")
                nc.scalar.copy(den[:], po[HD:HD + 1, :])
                recip = nrm_pool.tile([1, 512], F32, name=f"rc{h}{t}",
                                      tag="recip")
                nc.vector.reciprocal_approx_fast(recip[:], den[:])
                rfac = nrm_pool.tile([HD, 512], F32, name=f"rf{h}{t}",
                                     tag="rfac")
                nc.gpsimd.partition_broadcast(rfac[:], recip[:])
                nc.vector.tensor_mul(
                    attnTs[h // 2][64 * (h % 2):64 * (h % 2) + HD,
                                   512 * t:512 * (t + 1)],
                    po[0:HD, :], rfac[:])

            def att_jobs_for_pair(t, pair):
                """Emission thunks for one head-pair's q-tile t (2 chains
                block-interleaved, then the normalizations)."""
                jobs = []
                pos = {}
                nb = 4 * t + 4

                def alloc(pair=pair, t=t):
                    for h in pair:
                        pos[h] = po_pool.tile([128, 512], F32,
                                              name=f"po{h}{t}", tag="po")
                jobs.append(alloc)
                for b in range(nb):
                    for h in pair:
                        jobs.append(lambda h=h, b=b: attention_block(
                            h, t, b, nb, pos[h]))
                for h in pair:
                    jobs.append(lambda h=h: finish_tile(h, t, pos[h]))
                return jobs

            def send_a2a(i):
                for r in range(N_CORES):
                    nc.sync.dma_start(a2a_in[i][r],
                                      attnTs[i][:, 256 * r:256 * (r + 1)])
                nc.gpsimd.collective_compute(
                    "AllToAll", mybir.AluOpType.bypass,
                    replica_groups=[list(range(N_CORES))],
                    ins=[a2a_in[i][:]], outs=[a2a_out[i][:]])

            for sq in range(4):
                s0 = 512 * sq
                queue = (att_jobs_for_pair(sq - 1, (0, 1)) +
                         att_jobs_for_pair(sq - 1, (2, 3))) if sq else []
                qi = 0
                pq = [pacc_pool.tile([128, 512], F32, name=f"pq{sq}_{m}",
                                     tag="pacc") for m in range(2)]
                pkv = pacc_pool.tile([128, 512], F32, name=f"pkv{sq}",
                                     tag="pacc")
                for kc in range(16):
                    xtb = xtb_pool.tile([128, 512], BF16,
                                        name=f"xtb{sq}_{kc}", tag="xtb")
                    nc.sync.dma_start(xtb[:], xT_d.ap()[sq, kc])
                    st, sp = (kc == 0), (kc == 15)
                    for m in range(2):
                        nc.tensor.matmul(
                            pq[m][:], wq_sb[:, kc, 128 * m:128 * (m + 1)],
                            xtb[:], start=st, stop=sp)
                    nc.tensor.matmul(pkv[:], wkv_sb[:, kc, :], xtb[:],
                                     start=st, stop=sp)
                    # drain a slice of last quarter's attention jobs
                    want = (len(queue) * (kc + 1)) // 16
                    while qi < want:
                        queue[qi]()
                        qi += 1
                while qi < len(queue):
                    queue[qi]()
                    qi += 1
                # rope q -> qTs ; rope k -> kT ; copy v -> vT ; transpose V
                cs = cos_sb[:, s0:s0 + 512]
                sn = sin_sb[:, s0:s0 + 512]
                for m in range(2):
                    for hh in range(2):
                        rope_pair(qTs[2 * m + hh], (s0, s0 + 512), pq[m],
                                  64 * hh, cs, sn, f"q{sq}{m}{hh}")
                rope_pair(kT, (s0, s0 + 512), pkv, 0, cs, sn, f"k{sq}")
                nc.scalar.copy(vT[0:HD, s0:s0 + 512], pkv[64:128, :])
                for sc in range(4 * sq, 4 * sq + 4):
                    pvt = psc_pool.tile([128, 512], F32, name=f"pvt{sc}",
                                        tag="psc")
                    nc.tensor.transpose(pvt[:, 0:HD + 1],
                                        vT[:, 128 * sc:128 * (sc + 1)],
                                        identity[:])
                    nc.scalar.copy(v128[:, sc, 0:HD + 1], pvt[:, 0:HD + 1])

            nc.scalar.dma_start(wo_sb[:], wo_d.ap())

            # final q-tile (t=3): heads 0/1, ship their a2a chunk, heads 2/3
            for job in att_jobs_for_pair(3, (0, 1)):
                job()
            send_a2a(0)
            for job in att_jobs_for_pair(3, (2, 3)):
                job()
            send_a2a(1)

        # Stage W: out rows = attn_fullT.T @ wo, accumulated in two passes
        # (even h-chunks from a2a chunk 0, odd from chunk 1).
        with ExitStack() as ctx:
            af_pool = ctx.enter_context(tc.tile_pool(name="af", bufs=1))
            pw_pool = ctx.enter_context(
                tc.tile_pool(name="pw", bufs=1, space="PSUM"))
            osb_pool = ctx.enter_context(tc.tile_pool(name="osb", bufs=2))
            afs = []
            for i in range(2):
                af = af_pool.tile([128, N_CORES, ROWS_PER_CORE], BF16,
                                  name=f"attn_full{i}", uniquify=False)
                nc.sync.dma_start(af[:],
                                  a2a_out[i][:].rearrange("r p s -> p r s"))
                afs.append(af)
            pw = [[pw_pool.tile([128, 512], F32, name=f"pw{m}{n}",
                                tag=f"pw{m}{n}") for n in range(4)]
                  for m in range(2)]
            for i in range(2):          # a2a chunk: even then odd h-chunks
                for r in range(N_CORES):
                    kc = 2 * r + i
                    st = (i == 0 and r == 0)
                    sp = (i == 1 and r == N_CORES - 1)
                    for m in range(2):
                        lhs = afs[i][:, r, 128 * m:128 * (m + 1)]
                        for n in range(4):
                            nc.tensor.matmul(
                                pw[m][n][:], lhs,
                                wo_sb[:, kc, 512 * n:512 * (n + 1)],
                                start=st, stop=sp)
            for m in range(2):
                osb = osb_pool.tile([128, D], F32, name=f"osb{m}", tag="osb")
                for n in range(4):
                    nc.scalar.copy(osb[:, 512 * n:512 * (n + 1)], pw[m][n][:])
                nc.sync.dma_start(out_d.ap()[128 * m:128 * (m + 1), :], osb[:])

    nc.compile()
    return nc


_NC_CACHE = None
LAST_RESULT = None


def _get_nc():
    global _NC_CACHE
    if _NC_CACHE is None:
        _NC_CACHE = _build()
    return _NC_CACHE


def _permute_rope_cols(w):
    """Per-head column permutation: [d0,d1,...,d63] -> [evens..., odds...]."""
    Din, HDall = w.shape
    H = HDall // HD
    return np.ascontiguousarray(
        w.reshape(Din, H, HD // 2, 2).transpose(0, 1, 3, 2).reshape(Din, HDall))


def kernel(x, wq, wk, wv, wo, freqs_cos, freqs_sin, mask, start_pos=0):
    assert int(start_pos) == 0, "kernel specialized for start_pos == 0"
    import ml_dtypes
    x = np.asarray(x, np.float32)
    b, s, d = x.shape
    assert (b, s, d) == (1, S, D)
    xT = np.ascontiguousarray(x[0].T).astype(ml_dtypes.bfloat16)
    # pre-tile: xT[sq, kc] = contiguous (128, 512) block -> 1-descriptor DMAs
    xTt = np.ascontiguousarray(
        xT.reshape(16, 128, 4, 512).transpose(2, 0, 1, 3))
    wq_p = _permute_rope_cols(np.asarray(wq, np.float32))
    wk_p = _permute_rope_cols(np.asarray(wk, np.float32))
    wv = np.asarray(wv, np.float32)
    wot = np.ascontiguousarray(
        np.asarray(wo, np.float32).reshape(16, 128, D).transpose(1, 0, 2)
    ).astype(ml_dtypes.bfloat16)
    cosT = np.ascontiguousarray(np.asarray(freqs_cos, np.float32).T)
    sinT = np.ascontiguousarray(np.asarray(freqs_sin, np.float32).T)
    maskT01 = np.ascontiguousarray(
        (np.asarray(mask, np.float32)[:128, :128].T == 0.0)
    ).astype(ml_dtypes.bfloat16)

    in_maps = []
    for c in range(N_CORES):
        in_maps.append({
            "xT": xTt,
            "wq": np.ascontiguousarray(
                wq_p[:, QCOLS * c:QCOLS * (c + 1)].reshape(16, 128, QCOLS)
                .transpose(1, 0, 2)).astype(ml_dtypes.bfloat16),
            "wkv": np.ascontiguousarray(np.concatenate(
                [wk_p[:, HD * c:HD * (c + 1)], wv[:, HD * c:HD * (c + 1)]],
                axis=1).reshape(16, 128, KVCOLS)
                .transpose(1, 0, 2)).astype(ml_dtypes.bfloat16),
            "wo": wot,
            "cosT": cosT,
            "sinT": sinT,
            "maskT01": maskT01,
        })

    nc = _get_nc()
    res = bass_utils.run_bass_kernel_spmd(
        nc, in_maps, core_ids=list(range(N_CORES)),
        trace=bool(os.environ.get("BASS_TRACE")))
    global LAST_RESULT
    LAST_RESULT = res
    rows = [res.results[c]["out"] for c in range(N_CORES)]
    return np.concatenate(rows, axis=0).reshape(1, S, D).astype(np.float32)
